# revision 38
# baseline (speedup 1.0000x reference)
"""NetsVocab per-word MLP kernel for 8 Trainium2 NeuronCores.

Math (per active word w of 16, per box b of 8192):
    h1 = relu(x @ W1[w] + b1[w])        # [B,4096] @ [4096,64]
    h2 = relu(h1 @ W2[w] + b2[w])       # [B,64] @ [64,32]
    l  = h2 @ W3[w] + b3[w]             # [B,32] @ [32]
    out[b] = prod_w sigmoid(l[w,b])

Strategy: data-parallel over boxes (1024 per core), the 16 active words'
weights gathered on host and replicated to all cores; no collectives.
Layer 1 dominates (8.6 GFLOP/core) and runs MIXED PRECISION: 22 k-tiles
as bf16 PE matmuls + the last 10 k-tiles as fp8e4 DoubleRow matmuls
(each consumes TWO 128-deep k-slabs per 512-cycle pass - a true 2x on
this hardware), all accumulating into one fp32 PSUM chain. The whole
layer runs in x64 units: 64*W1 is exact in bf16 (exponent shift), puts
U(-1/64,1/64) weights in e4m3's normal range, and W2/64 undoes it
exactly after the relu since relu(64z) = 64 relu(z). Measured max rel
err 1.74e-2 vs the 2e-2 gate (fp8 k-tile count chosen from a CPU
bit-exact sweep; the error tail is dense, so the margin is robust).
Layers 2/3 are block-diagonal matmuls zero-padded to 128 lhsT columns
(FWL-eligible); L3 packs FOUR words per matmul for m-pairs (0,1),(2,3),
(4,5) with logits on partitions {0,1,32,33}. The 16-sigmoid product
accumulates into a [2, BC] running-product tile; partition 1 combines
via one SBUF->SBUF DMA bounce (engines only address base partitions
{0,32,64,96}). A duplicate word in `words` is packed as tile 7 so the
tail needs one sigmoid applied twice instead of two sigmoids.

Perf structure (per core, ~123 us, was 147 us all-bf16):
  - host packs everything partition-major: each DMA descriptor is a
    multi-KiB contiguous per-partition run; the ~358 GB/s/core wire is
    the binding constraint early on, so the two HWDGE queues are
    byte-balanced (scalar: xT waves + fp8 tensors, sync: w1 half 0
    waves + half 1) and ordered by first-use time
  - a short warmup chain bridges engine boot (~8 us) to the first fp8
    data; pass 1 then opens with the fp8 DoubleRow phase - real PE work
    needing only 2.5 MiB of wire - which covers the HAM cold-clock
    window (the clock gate needs ~4.5 us of CONTINUOUS PE activity to
    raise the core, PE and DMA both, from half clock)
  - pass 1 holds 6 PSUM accumulators ({m0-2}x{n0,n1}), k-outer, paced
    at 1.28 us/k-tile = 300 GB/s demand, just under the wire
  - epilogues run as a 3-stage pipeline, one stage per tick between
    k-matmuls (the paired stage-c issues two sigmoids, so anything
    denser overruns the strict-FIFO ACT queue and stalls the PE)
  - after the last real matmul, dummy "warmdown" matmuls keep the PE
    busy through the serial ACT/DVE/DMA tail so HAM does not halve the
    clock under it; the tail epilogue is monolithic (ACT ops cost
    ~260ns + 0.83ns/col, so fewer wide ops beat many narrow ones)

Layouts (per core):
    xT   [128, 22, 1024] bf16   [p, k, b] <-> x[c*1024+b, k*128+p]
    w1   [2, 128, 22, 512] bf16 [half, p, k, c] <-> 64*W1cat[k*128+p, .]
    xf8  [128, 5, 2, 1024] f8e4 DoubleRow pairs, k-tiles 22..31
    w1f8 [128, 5, 2, 1024] f8e4
    w2   [128, 8, 128] bf16     per m-tile block-diag /64, zero-padded
    w3   [64, 8, 128] bf16      m6/m7 blocks; w3q [128, 3, 128] 4-word
    b1   [128, 8] f32 (x64), b2 [64, 8], b3 [33, 8], b3q [34, 3]
    out  [1, 1024] f32
"""

import os

import numpy as np
import ml_dtypes

import concourse.bass as bass
import concourse.tile as tile
from concourse import bacc
from concourse import mybir
from concourse.bass import ts
from concourse.bass_utils import run_bass_kernel_spmd

BF16 = mybir.dt.bfloat16
F32 = mybir.dt.float32
AF = mybir.ActivationFunctionType

N_CORES = 8
B = 8192            # total boxes
BC = B // N_CORES   # boxes per core (1024)
F = 4096            # features
NW = 16             # active words
H1 = 64
H2 = 32
KT = F // 128       # 32 k-tiles
KF = 5              # fp8 DoubleRow k-tile PAIRS (last 10 k-tiles)
KB = KT - 2 * KF    # 24 bf16 k-tiles
MT = NW * H1 // 128  # 8 m-tiles (wh = w*64+h, 2 words per tile)
NT = BC // 512      # 2 n-tiles of 512 boxes

LAST_RESULTS = None  # BassKernelResults of the most recent run (for test.py)


def build_nc(same7):
    """same7: tile 7 holds the same word twice (duplicate in `words`), so
    its two logits are identical and one sigmoid (applied twice) covers
    both - removes one ACT op from the serial tail chain."""
    nc = bacc.Bacc("TRN2", target_bir_lowering=False, debug=False)

    F8 = mybir.dt.float8e4
    xT_d = nc.dram_tensor("xT", [128, KB, BC], BF16, kind="ExternalInput")
    w1_d = nc.dram_tensor("w1", [2, 128, KB, 512], BF16, kind="ExternalInput")
    xf8_d = nc.dram_tensor("xf8", [128, KF, 2, BC], F8, kind="ExternalInput")
    w1f8_d = nc.dram_tensor("w1f8", [128, KF, 2, 1024], F8, kind="ExternalInput")
    w2_d = nc.dram_tensor("w2", [128, MT, 128], BF16, kind="ExternalInput")
    w3_d = nc.dram_tensor("w3", [64, MT, 128], BF16, kind="ExternalInput")
    w3q_d = nc.dram_tensor("w3q", [128, 3, 128], BF16, kind="ExternalInput")
    b3q_d = nc.dram_tensor("b3q", [34, 3], F32, kind="ExternalInput")
    b1_d = nc.dram_tensor("b1", [128, MT], F32, kind="ExternalInput")
    b2_d = nc.dram_tensor("b2", [64, MT], F32, kind="ExternalInput")
    b3_d = nc.dram_tensor("b3", [33, MT], F32, kind="ExternalInput")
    out_d = nc.dram_tensor("out", [1, BC], F32, kind="ExternalOutput")
    wsink_d = nc.dram_tensor("wsink", [1, 4], F32)
    wsink2_d = nc.dram_tensor("wsink2", [1, 4], F32)

    with tile.TileContext(nc) as tc:
        with (
            tc.tile_pool(name="big", bufs=1) as big,
            tc.tile_pool(name="smalls", bufs=1) as smalls,
            tc.tile_pool(name="h1p", bufs=8) as h1p,
            tc.tile_pool(name="h2p", bufs=8) as h2p,
            tc.tile_pool(name="sigp", bufs=4) as sigp,
            tc.tile_pool(name="prodp", bufs=1) as prodp,
            tc.tile_pool(name="accp", bufs=6, space="PSUM") as accp,
            tc.tile_pool(name="ps2p", bufs=1, space="PSUM") as ps2p,
            tc.tile_pool(name="ps3p", bufs=1, space="PSUM") as ps3p,
        ):
            w2_sb = smalls.tile([128, MT, 128], BF16, tag="w2", name="w2_sb")
            w3_sb = smalls.tile([64, MT, 128], BF16, tag="w3", name="w3_sb")
            w3q_sb = smalls.tile([128, 3, 128], BF16, tag="w3q", name="w3q_sb")
            b1_sb = smalls.tile([128, MT], F32, tag="b1", name="b1_sb")
            b2_sb = smalls.tile([64, MT], F32, tag="b2", name="b2_sb")
            b3_sb = smalls.tile([33, MT], F32, tag="b3", name="b3_sb")
            b3q_sb = smalls.tile([34, 3], F32, tag="b3q", name="b3q_sb")

            xT_sb = big.tile([128, KB, BC], BF16, tag="xT", name="xT_sb")
            w1_sb = big.tile([128, 2, KB, 512], BF16, tag="w1", name="w1_sb")
            xf8_sb = big.tile([128, KF, 2, BC], F8, tag="xf8", name="xf8_sb")
            w1f8_sb = big.tile([128, KF, 2, 1024], F8, tag="w1f8", name="w1f8_sb")

            # Front-loaded chunk waves: k0 alone so the very first k-tile
            # lands ~2 us after the triggers fire, then growing chunks
            # (bigger per-partition descriptor runs) once latency no
            # longer matters. Mid-kernel the wire (~358 GB/s/core) is the
            # binding constraint and pass 1's 300 GB/s demand fits under
            # it. xT rides the scalar-engine HWDGE queue, w1 the sync
            # queue: two independent descriptor generators and rings.
            # fp8 k-tiles FIRST: only 2.5 MiB total, and pass 1 starts
            # with the DoubleRow phase - real PE work that needs no
            # further wire, covering the cold-clock window while the
            # bf16 waves stream in behind it.
            for t in range(KF):
                nc.sync.dma_start(out=w1f8_sb[:, t], in_=w1f8_d[:, t])
                nc.scalar.dma_start(out=xf8_sb[:, t], in_=xf8_d[:, t])
            waves = [(0, 2), (2, 3), (5, 4), (9, 5), (14, 4),
                     (18, 4)]   # (k0, nk) over the KB bf16 k-tiles
            for wi, (k0, nk) in enumerate(waves):
                nc.sync.dma_start(
                    out=w1_sb[:, 0, k0:k0 + nk, :], in_=w1_d[0, :, k0:k0 + nk, :]
                )
                nc.scalar.dma_start(
                    out=xT_sb[:, k0:k0 + nk, :], in_=xT_d[:, k0:k0 + nk, :]
                )
                if wi == 0:
                    # SWDGE queue: keeps these many-small-descriptor loads
                    # off the HWDGE queue feeding the big streams.
                    nc.gpsimd.dma_start(out=w2_sb, in_=w2_d[:])
                    nc.gpsimd.dma_start(out=w3_sb, in_=w3_d[:])
                    nc.gpsimd.dma_start(out=w3q_sb, in_=w3q_d[:])
                    nc.gpsimd.dma_start(out=b1_sb, in_=b1_d[:])
                    nc.gpsimd.dma_start(out=b2_sb, in_=b2_d[:])
                    nc.gpsimd.dma_start(out=b3_sb, in_=b3_d[:])
                    nc.gpsimd.dma_start(out=b3q_sb, in_=b3q_d[:])
            for c0, cn in ((0, 6), (6, 5), (11, 6), (17, 5)):
                nc.sync.dma_start(
                    out=w1_sb[:, 1, c0:c0 + cn, :],
                    in_=w1_d[1, :, c0:c0 + cn, :],
                )

            # Warm up the PE's HAM clock gate during the initial DMA wait:
            # dummy matmuls on an uninitialized scratch tile (no producers,
            # so they schedule immediately) keep the PE CONTINUOUSLY busy
            # from boot (~8.4us) until k0 lands and the clock is warm
            # (~13.4us). The HAM clock-warm trigger needs ~4.5us of
            # continuous PE activity; any PE-idle gap in this window keeps
            # the whole core (PE and DMA) at half clock.
            warm_src = smalls.tile([128, 512], BF16, tag="warm", name="warm_src")
            nc.vector.memset(warm_src, 0.0)
            warm_ps = ps2p.tile([128, 512], F32, tag="ps2", name="warm_ps")
            NWARM = 4
            for wi in range(NWARM):
                nc.tensor.matmul(
                    warm_ps, warm_src[:, 0:128], warm_src,
                    start=(wi == 0), stop=(wi == NWARM - 1),
                )
            # Sink the warmup result to scratch DRAM so DCE can't drop the
            # accumulation chain.
            wsink = smalls.tile([1, 4], F32, tag="wsink", name="wsink")
            nc.vector.tensor_copy(wsink, warm_ps[0:1, 0:4])
            nc.sync.dma_start(out=wsink_d[:], in_=wsink)

            # Running product over the 8 word-pairs: prod[p, b] accumulates
            # prod_m sigmoid(logits) for pair-slot p (word 2m+p).
            prod = prodp.tile([2, BC], F32, tag="prod", name="prod")

            def l1_matmul(acc, m, n, k, start=False, stop=False):
                nc.tensor.matmul(
                    acc,
                    w1_sb[:, m // 4, k, ts(m % 4, 128)],
                    xT_sb[:, k, ts(n, 512)],
                    start=start,
                    stop=stop,
                )

            def l1_dr(acc, m, n, t, start=False, stop=False):
                # fp8 DoubleRow: one matmul consumes TWO k-tiles (two
                # k-slabs side-by-side in the free dim of both operands)
                # at ~2x the row rate. Weights are pre-scaled x64 on host
                # (undone exactly via W2/64) so U(-1/64,1/64) lands in
                # e4m3's normal range.
                nc.tensor.matmul(
                    acc,
                    w1f8_sb[:, t, :, ts(m, 128)],
                    xf8_sb[:, t, :, ts(n, 512)],
                    start=start,
                    stop=stop,
                    perf_mode=mybir.MatmulPerfMode.DoubleRow,
                )

            # Epilogue as a 3-stage pipeline. Each stage's cross-engine
            # producer gets a multi-k-tile head start before the PE reaches
            # the consuming matmul, so the in-order PE never waits on ACT.
            # W2/W3 are zero-padded to 128 lhsT columns: full-width weight
            # loads are FWL-eligible and pull ahead of in-flight matmuls;
            # narrow loads serialize (~300 ns each).
            def epi_a(e):
                m, n = e["m"], e["n"]
                h1_t = h1p.tile([128, 512], BF16, tag="h1", name=f"h1_{m}_{n}")
                nc.scalar.activation(
                    h1_t, e["acc"], AF.Relu, bias=b1_sb[:, m:m + 1]
                )
                e["h1"] = h1_t

            pair_h2 = {}

            def epi_b(e):
                m, n = e["m"], e["n"]
                ps2 = ps2p.tile([128, 512], F32, tag="ps2", name=f"ps2_{m}_{n}")
                nc.tensor.matmul(
                    ps2, w2_sb[:, m, :], e["h1"], start=True, stop=True
                )
                if m < 6:
                    # m0..m5 pair up (4 words per L3 matmul): even m writes
                    # h2 to rows 0:64 of a shared pair tile, odd m to rows
                    # 64:128; the odd epilogue's stage c consumes the pair.
                    t = m // 2
                    if m % 2 == 0:
                        pt = h2p.tile(
                            [128, 512], BF16, tag="h2", name=f"h2q_{t}_{n}"
                        )
                        pair_h2[(t, n)] = pt
                        dst = pt[0:H1, :]
                    else:
                        dst = pair_h2[(t, n)][H1:128, :]
                    nc.scalar.activation(
                        dst, ps2[0:H1, :], AF.Relu, bias=b2_sb[:, m:m + 1]
                    )
                    return
                h2_t = h2p.tile([H1, 512], BF16, tag="h2", name=f"h2_{m}_{n}")
                nc.scalar.activation(
                    h2_t, ps2[0:H1, :], AF.Relu, bias=b2_sb[:, m:m + 1]
                )
                e["h2"] = h2_t

            sig7 = {}

            def epi_c(e):
                m, n = e["m"], e["n"]
                if m < 6 and m % 2 == 0:
                    return   # pair completed by the odd sibling
                ps3 = ps3p.tile([128, 512], F32, tag="ps3", name=f"ps3_{m}_{n}")
                if m < 6:
                    # 4-word L3: logits land on partitions 0,1,32,33. Two
                    # narrow sigmoids (ACT may shift PSUM base 32 -> SBUF
                    # base 0; DVE needs equal SBUF base partitions).
                    t = m // 2
                    nc.tensor.matmul(
                        ps3, w3q_sb[:, t, :], pair_h2[(t, n)],
                        start=True, stop=True,
                    )
                    sig_lo = sigp.tile(
                        [2, 512], F32, tag="sig", name=f"sigqa_{t}_{n}"
                    )
                    nc.scalar.activation(
                        sig_lo, ps3[0:2, :], AF.Sigmoid,
                        bias=b3q_sb[0:2, t:t + 1],
                    )
                    sig_hi = sigp.tile(
                        [2, 512], F32, tag="sig", name=f"sigqb_{t}_{n}"
                    )
                    nc.scalar.activation(
                        sig_hi, ps3[32:34, :], AF.Sigmoid,
                        bias=b3q_sb[32:34, t:t + 1],
                    )
                    if t == 0:
                        nc.vector.tensor_mul(
                            prod[:, ts(n, 512)], sig_lo, sig_hi
                        )
                    else:
                        nc.vector.tensor_mul(
                            prod[:, ts(n, 512)], prod[:, ts(n, 512)], sig_lo
                        )
                        nc.vector.tensor_mul(
                            prod[:, ts(n, 512)], prod[:, ts(n, 512)], sig_hi
                        )
                    return
                nc.tensor.matmul(
                    ps3, w3_sb[:, m, :], e["h2"], start=True, stop=True
                )
                if m == 7:
                    # m=7's words are packed to lhsT cols 0 and 32, so
                    # their logits land on readable base partitions 0/32;
                    # they multiply into the final output directly instead
                    # of via prod + bounce. With same7 the two words are
                    # identical, so one sigmoid serves both.
                    sa = sigp.tile([1, 512], F32, tag="sig", name=f"s7a_{n}")
                    nc.scalar.activation(
                        sa, ps3[0:1, :], AF.Sigmoid, bias=b3_sb[0:1, 7:8]
                    )
                    if same7:
                        sig7[n] = (sa, sa)
                    else:
                        sb = sigp.tile([1, 512], F32, tag="sig", name=f"s7b_{n}")
                        nc.scalar.activation(
                            sb, ps3[32:33, :], AF.Sigmoid,
                            bias=b3_sb[32:33, 7:8],
                        )
                        sig7[n] = (sa, sb)
                    return
                sig_t = sigp.tile([2, 512], F32, tag="sig", name=f"sig_{m}_{n}")
                nc.scalar.activation(
                    sig_t, ps3[0:2, :], AF.Sigmoid, bias=b3_sb[0:2, m:m + 1]
                )
                nc.vector.tensor_mul(
                    prod[:, ts(n, 512)], prod[:, ts(n, 512)], sig_t
                )

            stage_q = []
            EPI_STAGES = (epi_a, epi_b, epi_c)

            def tick():
                # Advance the oldest pending epilogue by ONE stage. (One
                # stage per tick: the paired stage-c issues two sigmoids,
                # and advancing two stages per tick bunches more ACT work
                # than the tick spacing drains - the strict-FIFO ACT queue
                # then stalls the next epilogue matmul.)
                if not stage_q:
                    return
                e = stage_q[0]
                EPI_STAGES[e["s"]](e)
                e["s"] += 1
                if e["s"] == 3:
                    stage_q.pop(0)
                    on_done((e["m"], e["n"]))

            # Pass 1: k-outer over {m0,m1,m2} x {n0,n1} - 6 accumulators,
            # 6 matmuls per k-tile (1.28 us/k-tile warm), roughly pacing
            # the per-k-tile DMA arrival.
            P1 = [(0, 0), (0, 1), (1, 0), (1, 1), (2, 0), (2, 1)]
            accs = {
                mn: accp.tile(
                    [128, 512], F32, tag="acc", name=f"acc_p1_{mn[0]}_{mn[1]}"
                )
                for mn in P1
            }
            for t in range(KF):
                for mn in P1:
                    l1_dr(accs[mn], mn[0], mn[1], t, start=(t == 0))
            for k in range(KB):
                for mn in P1:
                    l1_matmul(accs[mn], mn[0], mn[1], k, stop=(k == KB - 1))
            for mn in P1:
                stage_q.append({"m": mn[0], "n": mn[1], "acc": accs[mn], "s": 0})

            TICKS = (1, 3, 5, 7, 9, 11, 13, 15, 18, 21)
            pre = {}

            def on_done(mn):
                # m6 completing finalizes prod for that n-half: bounce
                # partition 1 and pre-multiply, hidden under m7's k-loops.
                if mn in ((6, 0), (6, 1)):
                    n = mn[1]
                    r1 = prodp.tile([1, 512], F32, tag=f"r1_{n}", name=f"r1_{n}")
                    nc.sync.dma_start(out=r1, in_=prod[1:2, ts(n, 512)])
                    p = prodp.tile([1, 512], F32, tag=f"pre_{n}", name=f"pre_{n}")
                    nc.vector.tensor_mul(p, prod[0:1, ts(n, 512)], r1)
                    pre[n] = p
                elif mn in ((7, 0), (7, 1)):
                    n = mn[1]
                    sa, sb = sig7[n]
                    o1 = prodp.tile([1, 512], F32, tag=f"o1_{n}", name=f"o1_{n}")
                    nc.vector.tensor_mul(o1, pre[n], sa)
                    o2 = prodp.tile([1, 512], F32, tag=f"o2_{n}", name=f"o2_{n}")
                    nc.vector.tensor_mul(o2, o1, sb)
                    nc.sync.dma_start(out=out_d[:, ts(n, 512)], in_=o2)

            # m3..m6 as n0/n1 k-interleaved pairs (alternating accumulator
            # banks every matmul avoids same-bank PSUM write-queue
            # backpressure; the shared lhsT halves LDWEIGHTS traffic),
            # then m7 as two singles so n0's combine hides under the
            # (7,1) k-loop. The (7,1) epilogue itself is the tail: it
            # drains monolithically after the last matmul.
            for m in range(3, 7):
                acc0 = accp.tile([128, 512], F32, tag="acc", name=f"acc_{m}_0")
                acc1 = accp.tile([128, 512], F32, tag="acc", name=f"acc_{m}_1")
                for k in range(KB):
                    l1_matmul(acc0, m, 0, k, start=(k == 0))
                    l1_matmul(acc1, m, 1, k, start=(k == 0))
                    if k in TICKS:
                        tick()
                for t in range(KF):
                    l1_dr(acc0, m, 0, t, stop=(t == KF - 1))
                    l1_dr(acc1, m, 1, t, stop=(t == KF - 1))
                stage_q.append({"m": m, "n": 0, "acc": acc0, "s": 0})
                stage_q.append({"m": m, "n": 1, "acc": acc1, "s": 0})

            acc70 = accp.tile([128, 512], F32, tag="acc", name="acc_7_0")
            for k in range(KB):
                l1_matmul(acc70, 7, 0, k, start=(k == 0))
                if k in TICKS:
                    tick()
            for t in range(KF):
                l1_dr(acc70, 7, 0, t, stop=(t == KF - 1))
            stage_q.append({"m": 7, "n": 0, "acc": acc70, "s": 0})
            acc71 = accp.tile([128, 512], F32, tag="acc", name="acc_7_1")
            # Generous stage spacing for the final in-loop epilogue ((7,0)
            # + the prod combines) so the PE never stalls on ACT right
            # before the tail.
            for k in range(KB):
                l1_matmul(acc71, 7, 1, k, start=(k == 0))
                if k in (2, 6, 10, 14):
                    tick()
            for t in range(KF):
                l1_dr(acc71, 7, 1, t, stop=(t == KF - 1))
            stage_q.append({"m": 7, "n": 1, "acc": acc71, "s": 0})
            # Tail drain with the PE kept hot: HAM halves the core clock
            # (stretching the serial ACT/DVE/DMA tail chain ~2x) as soon
            # as the PE goes idle, so dummy matmuls fill the PE-idle
            # windows between the (7,1) epilogue stages and through the
            # final muls + output DMA.
            wd = accp.tile([128, 512], F32, tag="acc", name="wd_ps")

            def warmdown(cnt):
                for _ in range(cnt):
                    nc.tensor.matmul(
                        wd, warm_src[:, 0:128], warm_src, start=True, stop=True
                    )

            while stage_q:
                tick()
                warmdown(2)
            warmdown(8)
            wsink2 = smalls.tile([1, 4], F32, tag="wsink2", name="wsink2")
            nc.vector.tensor_copy(wsink2, wd[0:1, 0:4])
            # Sync queue, AFTER the output DMA (which fired inside the
            # drain loop): a gpsimd-queue DMA here adds a ~1.9us SWDGE
            # drain that gates teardown past the output landing.
            nc.sync.dma_start(out=wsink2_d[:], in_=wsink2)


    nc.compile()
    return nc


_NC_CACHE = {}


def _get_nc(same7):
    if same7 not in _NC_CACHE:
        _NC_CACHE[same7] = build_nc(same7)
    return _NC_CACHE[same7]


def _word_order(words):
    """Permutation of the 16 word positions; if some word value repeats,
    two of its positions go last (tile 7) so the kernel can share one
    sigmoid for that tile."""
    order = list(range(NW))
    vals = {}
    for i, w in enumerate(words.tolist()):
        vals.setdefault(w, []).append(i)
    for w, pos in vals.items():
        if len(pos) >= 2:
            a, b = pos[0], pos[1]
            order = [i for i in order if i not in (a, b)] + [a, b]
            return order, True
    return order, False


def _pack_inputs(x, words, W1, b1, W2, b2, W3, b3, order):
    bf = ml_dtypes.bfloat16
    e4 = ml_dtypes.float8_e4m3
    words = np.asarray(words).astype(np.int64)[order]
    KBF = KB * 128                                  # bf16 feature rows

    w1g = np.asarray(W1)[words]                     # [16, 4096, 64]
    # Whole layer-1 runs in x64 units: 64*W1 is exact in bf16 (exponent
    # shift), puts the weights in e4m3's normal range for the fp8
    # k-tiles, and is undone exactly by W2/64 after the relu.
    w1cat = w1g.transpose(1, 0, 2).reshape(F, NW * H1) * 64.0  # [4096, 1024]
    # -> [half, p, k, col]: partition-major so each partition's whole
    # k-range is one contiguous DMA run.
    w1p = np.ascontiguousarray(
        w1cat[:KBF].astype(bf).reshape(KB, 128, 2, 512).transpose(2, 1, 0, 3)
    )                                               # [2, 128, KB, 512]
    w1f8 = np.ascontiguousarray(
        w1cat[KBF:].astype(e4).reshape(KF, 2, 128, NW * H1)
        .transpose(2, 0, 1, 3)
    )                                               # [128, KF, 2, 1024]
    b1cat = np.asarray(b1)[words].reshape(NW * H1) * 64.0  # [1024]
    b1p = np.ascontiguousarray(b1cat.reshape(MT, 128).T).astype(np.float32)

    w2g = np.asarray(W2)[words] / 64.0              # [16, 64, 32]
    w2blk = np.zeros((MT, 128, 128), np.float32)
    for t in range(MT):
        w2blk[t, 0:64, 0:32] = w2g[2 * t]
        w2blk[t, 64:128, 32:64] = w2g[2 * t + 1]
    w2p = np.ascontiguousarray(w2blk.transpose(1, 0, 2)).astype(bf)  # [128,8,64]
    b2g = np.asarray(b2)[words]                     # [16, 32]
    b2blk = np.zeros((MT, 64), np.float32)
    for t in range(MT):
        b2blk[t, 0:32] = b2g[2 * t]
        b2blk[t, 32:64] = b2g[2 * t + 1]
    b2p = np.ascontiguousarray(b2blk.T).astype(np.float32)           # [64, 8]

    w3g = np.asarray(W3)[words]                     # [16, 32]
    # 4-word L3 blocks for m-pairs (0,1),(2,3),(4,5): words 4t..4t+3 on
    # h2 rows {0,32,64,96}+32, logits to cols {0,1,32,33}.
    w3qblk = np.zeros((3, 128, 128), np.float32)
    b3q = np.zeros((34, 3), np.float32)
    for t in range(3):
        for j, col in enumerate((0, 1, 32, 33)):
            w3qblk[t, 32 * j:32 * j + 32, col] = w3g[4 * t + j]
            b3q[col, t] = np.asarray(b3)[words][4 * t + j]
    w3qp = np.ascontiguousarray(w3qblk.transpose(1, 0, 2)).astype(bf)
    w3blk = np.zeros((MT, 64, 128), np.float32)
    for t in range(MT):
        w3blk[t, 0:32, 0] = w3g[2 * t]
        # m=7's odd word goes to col 32 so its logit lands on a readable
        # base partition for the split-sigmoid tail path.
        w3blk[t, 32:64, 32 if t == MT - 1 else 1] = w3g[2 * t + 1]
    w3p = np.ascontiguousarray(w3blk.transpose(1, 0, 2)).astype(bf)  # [64, 8, 2]
    b3g = np.asarray(b3)[words]                     # [16]
    b3blk = b3g.reshape(MT, 2)
    b3p = np.zeros((33, MT), np.float32)
    b3p[0:2, :] = b3blk.T
    b3p[32, :] = b3blk[:, 1]

    x = np.asarray(x, dtype=np.float32)
    shared = {"w1": w1p, "w1f8": w1f8, "w2": w2p, "w3": w3p,
              "w3q": w3qp, "b3q": b3q, "b1": b1p, "b2": b2p, "b3": b3p}
    in_maps = []
    for c in range(N_CORES):
        xc = x[c * BC:(c + 1) * BC, :]
        # [p, k, b] partition-major (one contiguous 48 KiB run per partition)
        xT_c = np.ascontiguousarray(
            xc[:, :KBF].astype(bf).T.reshape(KB, 128, BC).transpose(1, 0, 2)
        )
        xf8_c = np.ascontiguousarray(
            xc[:, KBF:].astype(e4).T.reshape(KF, 2, 128, BC)
            .transpose(2, 0, 1, 3)
        )                                           # [128, KF, 2, BC]
        in_maps.append({"xT": xT_c, "xf8": xf8_c, **shared})
    return in_maps


def _enable_trace():
    """Register the axon NTFF profile hook (the image's antenv lacks
    axon_hooks, so boot degraded silently) and disable artifact upload."""
    import sys
    import types
    import antenv
    from concourse import bass_utils as bu

    if "antenv.axon_hooks" not in sys.modules:
        mod = types.ModuleType("antenv.axon_hooks")
        mod._hook = None

        def set_axon_ntff_profile_hook(h):
            mod._hook = h

        def get_axon_ntff_profile_hook():
            return mod._hook

        mod.set_axon_ntff_profile_hook = set_axon_ntff_profile_hook
        mod.get_axon_ntff_profile_hook = get_axon_ntff_profile_hook
        sys.modules["antenv.axon_hooks"] = mod
        antenv.axon_hooks = mod

        from trn_agent_boot.trn_boot import _ntff_profile_via_ctypes

        set_axon_ntff_profile_hook(
            _ntff_profile_via_ctypes("/opt/axon/libaxon_pjrt.so")
        )

    bu.upload_artifacts = lambda tmpdir: tmpdir


def kernel(nBBox, x, words, W1, b1, W2, b2, W3, b3):
    global LAST_RESULTS
    words = np.asarray(words)
    order, same7 = _word_order(words)
    nc = _get_nc(same7)
    in_maps = _pack_inputs(x, words, W1, b1, W2, b2, W3, b3, order)
    trace = bool(int(os.environ.get("KERNEL_TRACE", "0")))
    if trace:
        _enable_trace()
    res = run_bass_kernel_spmd(
        nc, in_maps, core_ids=list(range(N_CORES)), trace=trace
    )
    LAST_RESULTS = res
    out = np.concatenate(
        [res.results[c]["out"].reshape(BC) for c in range(N_CORES)]
    )
    return out.astype(np.float32)[:, None]
